# revision 26
# baseline (speedup 1.0000x reference)
"""Trainium2 Bass kernel for a char-CNN (embed lookup + conv1d(K=5,pad=2) + bias + maxpool).

Math: out[n, f] = max_w ( b[f] + sum_k sum_d  E[ids[n, w+k-2], d] * Wc[f, d, k] )

Strategy (pure data-parallel over 8 cores, 4096 tokens each):
  * Host-side constant folding (weights only): G[k][v, f] = sum_d E[v, d] * Wc[f, d, k].
    The embedding+conv collapses to y[n,:,w] = sum_k G[k][ids[n,w+k-2], :] + b.
  * On device, table lookup runs on the TensorEngine as one-hot matmuls with
    contraction over the vocab (96) plus a constant-ones row that carries the bias:
      - ids (bf16, exact for 0..95) broadcast across 96 partitions by the
        otherwise-idle GpSimd engine's partition_broadcast custom instruction,
        keeping the TensorEngine free for tap matmuls only
      - one-hot = is_equal(bcast, iota_per_partition) on VectorE, written fp16
        into a padded [vocab+1, tokens, W+4] layout (dense writes); the 5 tap
        reads are shifted views with token boundaries seeing zeros
      - 5 taps x 2 precision splits PSUM-accumulated against G tables stored as
        fp16 hi + lo (hi+lo recovers ~22 mantissa bits); fp16 weights padded to
        128 columns keep LDWEIGHTS (fast-weight-load) hidden under the matmuls
      - reduce_max over the 16 positions on VectorE (PSUM is DVE-only)
  * The broadcast/one-hot for unit u+1 is emitted before unit u's taps so the
    in-order PE queue never stalls on the VectorE.
  * Output is produced as [group, F, 512] per core; host transposes/concats.
"""

import numpy as np

import concourse.bass as bass
import concourse.bacc as bacc
import concourse.mybir as mybir
from concourse.tile import TileContext
from concourse.bass_utils import run_bass_kernel_spmd

# Problem shapes (hardcoded per contract)
N, W = 32768, 16
VOCAB, D, F, K = 96, 100, 100, 5
N_CORES = 8
NSH = N // N_CORES            # tokens per core = 4096
UNIT = 64                     # tokens per pipeline unit (=> 1024 one-hot cols)
NUNIT = NSH // UNIT           # 64
GROUP = 512                   # tokens per ids DMA
NGROUP = NSH // GROUP         # 8
UPG = GROUP // UNIT           # units per group = 8
VP = VOCAB + 1                # 96 vocab rows + 1 ones row (bias)
WP = W + 4                    # padded char positions per token
FP = 128                      # F padded to 128 weight columns (enables FWL)

bf16 = mybir.dt.bfloat16
f16 = mybir.dt.float16
f32 = mybir.dt.float32
i32 = mybir.dt.int32


def build_nc():
    nc = bacc.Bacc("TRN2", target_bir_lowering=False)

    ids_d = nc.dram_tensor("ids", [NSH, W], i32, kind="ExternalInput")
    # G split tables: [v, (split s, tap k), f_padded]  s=0 -> fp16(G), s=1 -> fp16(G - hi)
    gtab_d = nc.dram_tensor("gtab", [VP, 2 * K, FP], f16, kind="ExternalInput")
    iota_d = nc.dram_tensor("iota", [VOCAB, 1], f32, kind="ExternalInput")
    out_d = nc.dram_tensor("out", [NGROUP, F, GROUP], f32, kind="ExternalOutput")

    with TileContext(nc) as tc:
        with (
            tc.tile_pool(name="consts", bufs=1) as consts,
            tc.tile_pool(name="outp", bufs=2) as outp,
            tc.tile_pool(name="idsp", bufs=3) as idsp,
            tc.tile_pool(name="bcp", bufs=3) as bcp,
            tc.tile_pool(name="psA", bufs=1, space="PSUM") as psA,
            tc.tile_pool(name="psB", bufs=3, space="PSUM") as psB,
        ):
            ids_tiles = {}

            def load_ids(g):
                # a group's ids in partition 0, converted to bf16 (0..95 are
                # exact in bf16); casting DMAs must be gpsimd-initiated
                idst = idsp.tile([1, GROUP * W], bf16, tag="ids")
                nc.gpsimd.dma_start(
                    out=idst[0:1, :],
                    in_=ids_d[g * GROUP : (g + 1) * GROUP, :].rearrange(
                        "t w -> (t w)"
                    ).unsqueeze(0),
                )
                ids_tiles[g] = idst

            # startup-critical sync-queue order (~4us completion cadence per
            # DMA): iota #1 (DVE warm + is_equal scalar), group-0 ids #2 as
            # RAW int32 (no cast -> allowed on the fast HWDGE path; is_equal
            # compares int32 in0 against the f32 iota directly), gtab #3.
            iota_t = consts.tile([VOCAB, 1], f32)
            nc.sync.dma_start(out=iota_t, in_=iota_d[:, :])
            ids0_i = idsp.tile([1, GROUP * W], i32, tag="ids_i")
            nc.sync.dma_start(
                out=ids0_i[0:1, :],
                in_=ids_d[0:GROUP, :].rearrange("t w -> (t w)").unsqueeze(0),
            )
            load_ids(1)
            # touch DVE with its steady-state opcodes early: absorbs the
            # engine's first-dispatch latency during the init phase.
            dve_warm = consts.tile([VOCAB, 2], f32, tag="dve_warm")
            nc.vector.tensor_scalar(
                out=dve_warm[:, 0:1],
                in0=iota_t[:, :],
                scalar1=iota_t[:, 0:1],
                scalar2=None,
                op0=mybir.AluOpType.is_equal,
            )
            nc.vector.reduce_max(
                out=dve_warm[:, 1:2],
                in_=iota_t[:, :],
                axis=mybir.AxisListType.X,
            )

            # Two persistent one-hot tiles, padded layout [VP, UNIT, W+4]:
            # char position w at column w+2, pad columns {0,1,18,19} stay zero,
            # row 96 constant 1.0 (bias row, consumed only by center tap).
            o_tiles = []
            s_tiles = []
            for j in range(3):
                ot = consts.tile([VP, UNIT, WP], f16, tag=f"onehot{j}")
                # zero only the pad columns once; is_equal rewrites the real
                # positions every unit, pads stay zero forever.
                nc.vector.memset(ot[0:VOCAB, :, 0:2], 0.0)
                nc.vector.memset(ot[0:VOCAB, :, 2 + W : WP], 0.0)
                nc.vector.memset(ot[VOCAB : VOCAB + 1, :, :], 1.0)
                o_tiles.append(ot)
                # shifted copy (w+1), maintained by the idle Scalar engine so
                # odd taps read 4-byte-aligned fp16 offsets
                st = consts.tile([VP, UNIT, WP], f16, tag=f"oshift{j}")
                s_tiles.append(st)

            def bcast(u):
                # broadcast ids across 96 partitions on GpSimd + one-hot on DVE
                g, uu = divmod(u, UPG)
                if g == 0:
                    # group 0 rides the early HWDGE raw-i32 load
                    idst = ids0_i
                    bcz = bcp.tile([VOCAB, UNIT, W], i32, tag="bcast_i")
                else:
                    idst = ids_tiles[g]
                    bcz = bcp.tile([VOCAB, UNIT, W], bf16, tag="bcast")
                nc.gpsimd.partition_broadcast(
                    out_ap=bcz.rearrange("v t w -> v (t w)"),
                    in_ap=idst[0:1, uu * (UNIT * W) : (uu + 1) * (UNIT * W)],
                    channels=VOCAB,
                )
                # one-hot: O[v, t, w+2] = (ids[t, w] == v), dense write
                o_t = o_tiles[u % 3]
                nc.vector.tensor_scalar(
                    out=o_t[0:VOCAB, :, 2 : 2 + W],
                    in0=bcz[:, :, :],
                    scalar1=iota_t[:, 0:1],
                    scalar2=None,
                    op0=mybir.AluOpType.is_equal,
                )
                # shift-by-one copy for the odd taps (Scalar engine, off the
                # critical path)
                o_s = s_tiles[u % 3]
                nc.scalar.copy(
                    out=o_s[:, :, 0 : WP - 1],
                    in_=o_t[:, :, 1:WP],
                )

            gtab = consts.tile([VP, 2 * K, FP], f16)
            nc.sync.dma_start(
                out=gtab.rearrange("v s f -> v (s f)"),
                in_=gtab_d.rearrange("v s f -> v (s f)"),
            )

            bcast(0)
            bcast(1)

            # PE warmup: emitted after the prologue so the PE's activity window
            # stays busy right up to the first real matmul (HAM stays warm).
            warm = psA.tile([1, 1], f32, tag="warm")
            for _ in range(96):
                nc.tensor.matmul(
                    warm[0:1, 0:1],
                    iota_t[0:1, 0:1],
                    iota_t[0:1, 0:1],
                    start=True,
                    stop=True,
                )
            out_sb = None
            for u in range(NUNIT):
                g, uu = divmod(u, UPG)
                if uu == 0:
                    out_sb = outp.tile([F, GROUP], f32, tag="osb")
                    if 1 < g + 2 < NGROUP:
                        load_ids(g + 2)
                # emit bcast+one-hot two units ahead of this unit's taps so
                # the in-order PE queue never stalls on GpSimd/DVE latency.
                if u + 2 < NUNIT:
                    bcast(u + 2)

                o_t = o_tiles[u % 3]
                o_s = s_tiles[u % 3]
                # 5 taps x 2 precision splits, PSUM-accumulated (N=512 each).
                # Odd taps read the shifted tile at even (4B-aligned) offsets.
                ys = [psB.tile([FP, 32, W], f32, tag=f"y{h}", name=f"y{h}") for h in range(2)]
                first = True
                for s in range(2):
                    for k in range(K):
                        src_t, kk = (o_t, k) if (k % 2 == 0 or u == 0) else (o_s, k - 1)
                        for h in range(2):
                            nc.tensor.matmul(
                                ys[h][:, :, :],
                                gtab[:, s * K + k, :],
                                src_t[:, h * 32 : (h + 1) * 32, kk : kk + W],
                                start=first,
                                stop=(s == 1 and k == K - 1),
                                skip_group_check=True,
                            )
                        first = False

                # max over the 16 char positions (dense innermost reduce)
                for h in range(2):
                    nc.vector.reduce_max(
                        out=out_sb[:, uu * UNIT + h * 32 : uu * UNIT + (h + 1) * 32],
                        in_=ys[h][0:F, :, :],
                        axis=mybir.AxisListType.X,
                    )

                # stream this unit's result out immediately so the kernel
                # tail only waits on the final 64 tokens
                nc.sync.dma_start(
                    out=out_d[g, :, uu * UNIT : (uu + 1) * UNIT],
                    in_=out_sb[:, uu * UNIT : (uu + 1) * UNIT],
                )

    nc.compile()
    return nc


def make_consts(embed_table, conv_w, conv_b):
    # G[k][v, f] = sum_d E[v, d] * Wc[f, d, k] in float64, split hi/lo fp16
    G = np.einsum(
        "vd,fdk->kvf", embed_table.astype(np.float64), conv_w.astype(np.float64)
    )
    Gf = np.zeros((K, VP, F), np.float64)
    Gf[:, 0:VOCAB, :] = G
    Gf[2, VOCAB, :] = conv_b.astype(np.float64)  # bias rides center tap
    hi = Gf.astype(np.float32).astype(np.float16)
    lo = (Gf - hi.astype(np.float64)).astype(np.float32).astype(np.float16)
    gtab = np.zeros((VP, 2 * K, FP), np.float16)
    gtab[:, 0:K, 0:F] = np.transpose(hi, (1, 0, 2))
    gtab[:, K : 2 * K, 0:F] = np.transpose(lo, (1, 0, 2))
    iota = np.arange(VOCAB, dtype=np.float32).reshape(VOCAB, 1)
    return gtab, iota


_NC_CACHE = {}

# Test-harness knobs (ignored by normal kernel() use)
TRACE = False
LAST_RESULT = None


def kernel(char_ids, embed_table, conv_w, conv_b):
    global LAST_RESULT
    char_ids = np.asarray(char_ids)
    gtab, iota = make_consts(
        np.asarray(embed_table), np.asarray(conv_w), np.asarray(conv_b)
    )

    if "nc" not in _NC_CACHE:
        _NC_CACHE["nc"] = build_nc()
    nc = _NC_CACHE["nc"]

    in_maps = []
    for c in range(N_CORES):
        shard = np.ascontiguousarray(char_ids[c * NSH : (c + 1) * NSH])
        in_maps.append({"ids": shard, "gtab": gtab, "iota": iota})

    kwargs = {}
    if TRACE:
        kwargs = dict(trace=True, trace_cores=list(range(N_CORES)))
    res = run_bass_kernel_spmd(nc, in_maps, core_ids=list(range(N_CORES)), **kwargs)
    LAST_RESULT = res

    out = np.empty((N, F), np.float32)
    for c in range(N_CORES):
        o = res.results[c]["out"]  # [NGROUP, F, GROUP]
        out[c * NSH : (c + 1) * NSH] = o.transpose(0, 2, 1).reshape(NSH, F)
    return out


# revision 27
# speedup vs baseline: 1.0032x; 1.0032x over previous
"""Trainium2 Bass kernel for a char-CNN (embed lookup + conv1d(K=5,pad=2) + bias + maxpool).

Math: out[n, f] = max_w ( b[f] + sum_k sum_d  E[ids[n, w+k-2], d] * Wc[f, d, k] )

Strategy (pure data-parallel over 8 cores, 4096 tokens each):
  * Host-side constant folding (weights only): G[k][v, f] = sum_d E[v, d] * Wc[f, d, k].
    The embedding+conv collapses to y[n,:,w] = sum_k G[k][ids[n,w+k-2], :] + b.
  * On device, table lookup runs on the TensorEngine as one-hot matmuls with
    contraction over the vocab (96) plus a constant-ones row that carries the bias:
      - ids (bf16, exact for 0..95) broadcast across 96 partitions by the
        otherwise-idle GpSimd engine's partition_broadcast custom instruction,
        keeping the TensorEngine free for tap matmuls only
      - one-hot = is_equal(bcast, iota_per_partition) on VectorE, written fp16
        into a padded [vocab+1, tokens, W+4] layout (dense writes); the 5 tap
        reads are shifted views with token boundaries seeing zeros
      - 5 taps x 2 precision splits PSUM-accumulated against G tables stored as
        fp16 hi + lo (hi+lo recovers ~22 mantissa bits); fp16 weights padded to
        128 columns keep LDWEIGHTS (fast-weight-load) hidden under the matmuls
      - reduce_max over the 16 positions on VectorE (PSUM is DVE-only)
  * The broadcast/one-hot for unit u+1 is emitted before unit u's taps so the
    in-order PE queue never stalls on the VectorE.
  * Output is produced as [group, F, 512] per core; host transposes/concats.
"""

import numpy as np

import concourse.bass as bass
import concourse.bacc as bacc
import concourse.mybir as mybir
from concourse.tile import TileContext
from concourse.bass_utils import run_bass_kernel_spmd

# Problem shapes (hardcoded per contract)
N, W = 32768, 16
VOCAB, D, F, K = 96, 100, 100, 5
N_CORES = 8
NSH = N // N_CORES            # tokens per core = 4096
UNIT = 64                     # tokens per pipeline unit (=> 1024 one-hot cols)
NUNIT = NSH // UNIT           # 64
GROUP = 512                   # tokens per ids DMA
NGROUP = NSH // GROUP         # 8
UPG = GROUP // UNIT           # units per group = 8
VP = VOCAB + 1                # 96 vocab rows + 1 ones row (bias)
WP = W + 4                    # padded char positions per token
FP = 128                      # F padded to 128 weight columns (enables FWL)

bf16 = mybir.dt.bfloat16
f16 = mybir.dt.float16
f32 = mybir.dt.float32
i32 = mybir.dt.int32


def build_nc():
    nc = bacc.Bacc("TRN2", target_bir_lowering=False)

    ids_d = nc.dram_tensor("ids", [NSH, W], i32, kind="ExternalInput")
    # G split tables: [v, (split s, tap k), f_padded]  s=0 -> fp16(G), s=1 -> fp16(G - hi)
    gtab_d = nc.dram_tensor("gtab", [VP, 2 * K, FP], f16, kind="ExternalInput")
    iota_d = nc.dram_tensor("iota", [VOCAB, 1], f32, kind="ExternalInput")
    out_d = nc.dram_tensor("out", [NGROUP, F, GROUP], f32, kind="ExternalOutput")

    with TileContext(nc) as tc:
        with (
            tc.tile_pool(name="consts", bufs=1) as consts,
            tc.tile_pool(name="outp", bufs=2) as outp,
            tc.tile_pool(name="idsp", bufs=3) as idsp,
            tc.tile_pool(name="bcp", bufs=3) as bcp,
            tc.tile_pool(name="psA", bufs=1, space="PSUM") as psA,
            tc.tile_pool(name="psB", bufs=3, space="PSUM") as psB,
        ):
            ids_tiles = {}

            def load_ids(g):
                # all of a group's ids in partition 0, converted to bf16
                # (0..95 are exact in bf16); emitted ahead of the other
                # constant loads on the gpsimd queue (casting DMAs must be
                # gpsimd-initiated)
                idst = idsp.tile([1, GROUP * W], bf16, tag="ids")
                nc.gpsimd.dma_start(
                    out=idst[0:1, :],
                    in_=ids_d[g * GROUP : (g + 1) * GROUP, :].rearrange(
                        "t w -> (t w)"
                    ).unsqueeze(0),
                )
                ids_tiles[g] = idst

            load_ids(0)
            load_ids(1)

            iota_t = consts.tile([VOCAB, 1], f32)
            nc.sync.dma_start(out=iota_t, in_=iota_d[:, :])
            # touch DVE with its steady-state opcodes early: absorbs the
            # engine's first-dispatch latency during the init phase.
            dve_warm = consts.tile([VOCAB, 2], f32, tag="dve_warm")
            nc.vector.tensor_scalar(
                out=dve_warm[:, 0:1],
                in0=iota_t[:, :],
                scalar1=iota_t[:, 0:1],
                scalar2=None,
                op0=mybir.AluOpType.is_equal,
            )
            nc.vector.reduce_max(
                out=dve_warm[:, 1:2],
                in_=iota_t[:, :],
                axis=mybir.AxisListType.X,
            )

            # Two persistent one-hot tiles, padded layout [VP, UNIT, W+4]:
            # char position w at column w+2, pad columns {0,1,18,19} stay zero,
            # row 96 constant 1.0 (bias row, consumed only by center tap).
            o_tiles = []
            s_tiles = []
            for j in range(3):
                ot = consts.tile([VP, UNIT, WP], f16, tag=f"onehot{j}")
                # zero only the pad columns once; is_equal rewrites the real
                # positions every unit, pads stay zero forever.
                nc.vector.memset(ot[0:VOCAB, :, 0:2], 0.0)
                nc.vector.memset(ot[0:VOCAB, :, 2 + W : WP], 0.0)
                nc.vector.memset(ot[VOCAB : VOCAB + 1, :, :], 1.0)
                o_tiles.append(ot)
                # shifted copy (w+1), maintained by the idle Scalar engine so
                # odd taps read 4-byte-aligned fp16 offsets
                st = consts.tile([VP, UNIT, WP], f16, tag=f"oshift{j}")
                s_tiles.append(st)

            def bcast(u):
                # broadcast ids across 96 partitions on GpSimd + one-hot on DVE
                g, uu = divmod(u, UPG)
                idst = ids_tiles[g]
                bcz = bcp.tile([VOCAB, UNIT, W], bf16, tag="bcast")
                nc.gpsimd.partition_broadcast(
                    out_ap=bcz.rearrange("v t w -> v (t w)"),
                    in_ap=idst[0:1, uu * (UNIT * W) : (uu + 1) * (UNIT * W)],
                    channels=VOCAB,
                )
                # one-hot: O[v, t, w+2] = (ids[t, w] == v), dense write
                o_t = o_tiles[u % 3]
                nc.vector.tensor_scalar(
                    out=o_t[0:VOCAB, :, 2 : 2 + W],
                    in0=bcz[:, :, :],
                    scalar1=iota_t[:, 0:1],
                    scalar2=None,
                    op0=mybir.AluOpType.is_equal,
                )
                # shift-by-one copy for the odd taps (Scalar engine, off the
                # critical path)
                o_s = s_tiles[u % 3]
                nc.scalar.copy(
                    out=o_s[:, :, 0 : WP - 1],
                    in_=o_t[:, :, 1:WP],
                )

            gtab = consts.tile([VP, 2 * K, FP], f16)
            nc.sync.dma_start(
                out=gtab.rearrange("v s f -> v (s f)"),
                in_=gtab_d.rearrange("v s f -> v (s f)"),
            )

            bcast(0)
            bcast(1)

            # PE warmup: emitted after the prologue so the PE's activity window
            # stays busy right up to the first real matmul (HAM stays warm).
            warm = psA.tile([1, 1], f32, tag="warm")
            for _ in range(96):
                nc.tensor.matmul(
                    warm[0:1, 0:1],
                    iota_t[0:1, 0:1],
                    iota_t[0:1, 0:1],
                    start=True,
                    stop=True,
                )
            out_sb = None
            for u in range(NUNIT):
                g, uu = divmod(u, UPG)
                if uu == 0:
                    out_sb = outp.tile([F, GROUP], f32, tag="osb")
                    if g + 2 < NGROUP:
                        load_ids(g + 2)
                # emit bcast+one-hot two units ahead of this unit's taps so
                # the in-order PE queue never stalls on GpSimd/DVE latency.
                if u + 2 < NUNIT:
                    bcast(u + 2)

                o_t = o_tiles[u % 3]
                o_s = s_tiles[u % 3]
                # 5 taps x 2 precision splits, PSUM-accumulated (N=512 each).
                # Odd taps read the shifted tile at even (4B-aligned) offsets.
                ys = [psB.tile([FP, 32, W], f32, tag=f"y{h}", name=f"y{h}") for h in range(2)]
                first = True
                for s in range(2):
                    for k in range(K):
                        src_t, kk = (o_t, k) if (k % 2 == 0 or u == 0) else (o_s, k - 1)
                        for h in range(2):
                            nc.tensor.matmul(
                                ys[h][:, :, :],
                                gtab[:, s * K + k, :],
                                src_t[:, h * 32 : (h + 1) * 32, kk : kk + W],
                                start=first,
                                stop=(s == 1 and k == K - 1),
                                skip_group_check=True,
                            )
                        first = False

                # max over the 16 char positions (dense innermost reduce)
                for h in range(2):
                    nc.vector.reduce_max(
                        out=out_sb[:, uu * UNIT + h * 32 : uu * UNIT + (h + 1) * 32],
                        in_=ys[h][0:F, :, :],
                        axis=mybir.AxisListType.X,
                    )

                # stream this unit's result out immediately so the kernel
                # tail only waits on the final 64 tokens
                nc.sync.dma_start(
                    out=out_d[g, :, uu * UNIT : (uu + 1) * UNIT],
                    in_=out_sb[:, uu * UNIT : (uu + 1) * UNIT],
                )

    nc.compile()
    return nc


def make_consts(embed_table, conv_w, conv_b):
    # G[k][v, f] = sum_d E[v, d] * Wc[f, d, k] in float64, split hi/lo fp16
    G = np.einsum(
        "vd,fdk->kvf", embed_table.astype(np.float64), conv_w.astype(np.float64)
    )
    Gf = np.zeros((K, VP, F), np.float64)
    Gf[:, 0:VOCAB, :] = G
    Gf[2, VOCAB, :] = conv_b.astype(np.float64)  # bias rides center tap
    hi = Gf.astype(np.float32).astype(np.float16)
    lo = (Gf - hi.astype(np.float64)).astype(np.float32).astype(np.float16)
    gtab = np.zeros((VP, 2 * K, FP), np.float16)
    gtab[:, 0:K, 0:F] = np.transpose(hi, (1, 0, 2))
    gtab[:, K : 2 * K, 0:F] = np.transpose(lo, (1, 0, 2))
    iota = np.arange(VOCAB, dtype=np.float32).reshape(VOCAB, 1)
    return gtab, iota


_NC_CACHE = {}

# Test-harness knobs (ignored by normal kernel() use)
TRACE = False
LAST_RESULT = None


def kernel(char_ids, embed_table, conv_w, conv_b):
    global LAST_RESULT
    char_ids = np.asarray(char_ids)
    gtab, iota = make_consts(
        np.asarray(embed_table), np.asarray(conv_w), np.asarray(conv_b)
    )

    if "nc" not in _NC_CACHE:
        _NC_CACHE["nc"] = build_nc()
    nc = _NC_CACHE["nc"]

    in_maps = []
    for c in range(N_CORES):
        shard = np.ascontiguousarray(char_ids[c * NSH : (c + 1) * NSH])
        in_maps.append({"ids": shard, "gtab": gtab, "iota": iota})

    kwargs = {}
    if TRACE:
        kwargs = dict(trace=True, trace_cores=list(range(N_CORES)))
    res = run_bass_kernel_spmd(nc, in_maps, core_ids=list(range(N_CORES)), **kwargs)
    LAST_RESULT = res

    out = np.empty((N, F), np.float32)
    for c in range(N_CORES):
        o = res.results[c]["out"]  # [NGROUP, F, GROUP]
        out[c * NSH : (c + 1) * NSH] = o.transpose(0, 2, 1).reshape(NSH, F)
    return out


# revision 31
# speedup vs baseline: 1.0212x; 1.0179x over previous
"""Trainium2 Bass kernel for a char-CNN (embed lookup + conv1d(K=5,pad=2) + bias + maxpool).

Math: out[n, f] = max_w ( b[f] + sum_k sum_d  E[ids[n, w+k-2], d] * Wc[f, d, k] )

Strategy (pure data-parallel over 8 cores, 4096 tokens each):
  * Host-side constant folding (weights only): G[k][v, f] = sum_d E[v, d] * Wc[f, d, k].
    The embedding+conv collapses to y[n,:,w] = sum_k G[k][ids[n,w+k-2], :] + b.
  * On device, table lookup runs on the TensorEngine as one-hot matmuls with
    contraction over the vocab (96) plus a constant-ones row that carries the bias:
      - ids (bf16, exact for 0..95) broadcast across 96 partitions by the
        otherwise-idle GpSimd engine's partition_broadcast custom instruction,
        keeping the TensorEngine free for tap matmuls only
      - one-hot = is_equal(bcast, iota_per_partition) on VectorE, written fp16
        into a padded [vocab+1, tokens, W+4] layout (dense writes); the 5 tap
        reads are shifted views with token boundaries seeing zeros
      - 5 taps x 2 precision splits PSUM-accumulated against G tables stored as
        fp16 hi + lo (hi+lo recovers ~22 mantissa bits); fp16 weights padded to
        128 columns keep LDWEIGHTS (fast-weight-load) hidden under the matmuls
      - reduce_max over the 16 positions on VectorE (PSUM is DVE-only)
  * The broadcast/one-hot for unit u+1 is emitted before unit u's taps so the
    in-order PE queue never stalls on the VectorE.
  * Output is produced as [group, F, 512] per core; host transposes/concats.
"""

import numpy as np

import concourse.bass as bass
import concourse.bacc as bacc
import concourse.mybir as mybir
from concourse.tile import TileContext
from concourse.bass_utils import run_bass_kernel_spmd

# Problem shapes (hardcoded per contract)
N, W = 32768, 16
VOCAB, D, F, K = 96, 100, 100, 5
N_CORES = 8
NSH = N // N_CORES            # tokens per core = 4096
UNIT = 64                     # tokens per pipeline unit (=> 1024 one-hot cols)
NUNIT = NSH // UNIT           # 64
GROUP = 512                   # tokens per ids DMA
NGROUP = NSH // GROUP         # 8
UPG = GROUP // UNIT           # units per group = 8
VP = VOCAB + 1                # 96 vocab rows + 1 ones row (bias)
WP = W + 4                    # padded char positions per token
FP = 128                      # F padded to 128 weight columns (enables FWL)

bf16 = mybir.dt.bfloat16
f16 = mybir.dt.float16
f32 = mybir.dt.float32
i32 = mybir.dt.int32


def build_nc():
    nc = bacc.Bacc("TRN2", target_bir_lowering=False)

    ids_d = nc.dram_tensor("ids", [NSH, W], i32, kind="ExternalInput")
    # G split tables: [v, (split s, tap k), f_padded]  s=0 -> fp16(G), s=1 -> fp16(G - hi)
    gtab_d = nc.dram_tensor("gtab", [VP, 2 * K, FP], f16, kind="ExternalInput")
    iota_d = nc.dram_tensor("iota", [VOCAB, 1], f32, kind="ExternalInput")
    oones_d = nc.dram_tensor("oones", [1, UNIT * WP], f16, kind="ExternalInput")
    out_d = nc.dram_tensor("out", [NGROUP, F, GROUP], f32, kind="ExternalOutput")

    with TileContext(nc) as tc:
        with (
            tc.tile_pool(name="consts", bufs=1) as consts,
            tc.tile_pool(name="outp", bufs=2) as outp,
            tc.tile_pool(name="idsp", bufs=3) as idsp,
            tc.tile_pool(name="bcp", bufs=3) as bcp,
            tc.tile_pool(name="psA", bufs=1, space="PSUM") as psA,
            tc.tile_pool(name="psB", bufs=3, space="PSUM") as psB,
        ):
            ids_tiles = {}

            def load_ids(g):
                # all of a group's ids in partition 0, converted to bf16
                # (0..95 are exact in bf16); emitted ahead of the other
                # constant loads on the gpsimd queue (casting DMAs must be
                # gpsimd-initiated)
                idst = idsp.tile([1, GROUP * W], bf16, tag="ids")
                nc.gpsimd.dma_start(
                    out=idst[0:1, :],
                    in_=ids_d[g * GROUP : (g + 1) * GROUP, :].rearrange(
                        "t w -> (t w)"
                    ).unsqueeze(0),
                )
                ids_tiles[g] = idst

            load_ids(0)
            load_ids(1)

            iota_t = consts.tile([VOCAB, 1], f32)
            nc.sync.dma_start(out=iota_t, in_=iota_d[:, :])
            # touch DVE with its steady-state opcodes early: absorbs the
            # engine's first-dispatch latency during the init phase.
            dve_warm = consts.tile([VOCAB, 2], f32, tag="dve_warm")
            nc.vector.tensor_scalar(
                out=dve_warm[:, 0:1],
                in0=iota_t[:, :],
                scalar1=iota_t[:, 0:1],
                scalar2=None,
                op0=mybir.AluOpType.is_equal,
            )
            nc.vector.reduce_max(
                out=dve_warm[:, 1:2],
                in_=iota_t[:, :],
                axis=mybir.AxisListType.X,
            )

            # Two persistent one-hot tiles, padded layout [VP, UNIT, W+4]:
            # char position w at column w+2, pad columns {0,1,18,19} stay zero,
            # row 96 constant 1.0 (bias row, consumed only by center tap).
            o_tiles = []
            s_tiles = []
            for j in range(3):
                ot = consts.tile([VP, UNIT, WP], f16, tag=f"onehot{j}")
                # zero only the pad columns once; is_equal rewrites the real
                # positions every unit, pads stay zero forever.
                nc.vector.memset(ot[0:VOCAB, :, 0:2], 0.0)
                nc.vector.memset(ot[0:VOCAB, :, 2 + W : WP], 0.0)
                nc.sync.dma_start(
                    out=ot[VOCAB : VOCAB + 1, :, :].rearrange("v t w -> v (t w)"),
                    in_=oones_d[:, :],
                )
                o_tiles.append(ot)
                # shifted copy (w+1), maintained by the idle Scalar engine so
                # odd taps read 4-byte-aligned fp16 offsets
                st = consts.tile([VP, UNIT, WP], f16, tag=f"oshift{j}")
                s_tiles.append(st)

            def bcast(u):
                # broadcast ids across 96 partitions on GpSimd + one-hot on DVE
                g, uu = divmod(u, UPG)
                idst = ids_tiles[g]
                bcz = bcp.tile([VOCAB, UNIT, W], bf16, tag="bcast")
                nc.gpsimd.partition_broadcast(
                    out_ap=bcz.rearrange("v t w -> v (t w)"),
                    in_ap=idst[0:1, uu * (UNIT * W) : (uu + 1) * (UNIT * W)],
                    channels=VOCAB,
                )
                # one-hot: O[v, t, w+2] = (ids[t, w] == v), dense write
                o_t = o_tiles[u % 3]
                nc.vector.tensor_scalar(
                    out=o_t[0:VOCAB, :, 2 : 2 + W],
                    in0=bcz[:, :, :],
                    scalar1=iota_t[:, 0:1],
                    scalar2=None,
                    op0=mybir.AluOpType.is_equal,
                )
                # shift-by-one copy for the odd taps (Scalar engine, off the
                # critical path)
                o_s = s_tiles[u % 3]
                nc.scalar.copy(
                    out=o_s[:, :, 0 : WP - 1],
                    in_=o_t[:, :, 1:WP],
                )

            gtab = consts.tile([VP, 2 * K, FP], f16)
            nc.sync.dma_start(
                out=gtab.rearrange("v s f -> v (s f)"),
                in_=gtab_d.rearrange("v s f -> v (s f)"),
            )

            bcast(0)
            bcast(1)

            # No PE warmup: the first real matmul starts ~9us after any warmup
            # could finish (beyond the 3.4us HAM window), so warmups only
            # occupy the in-order PE queue without keeping the clock warm.
            out_sb = None
            for u in range(NUNIT):
                g, uu = divmod(u, UPG)
                if uu == 0:
                    out_sb = outp.tile([F, GROUP], f32, tag="osb")
                    if g + 2 < NGROUP:
                        load_ids(g + 2)
                # emit bcast+one-hot two units ahead of this unit's taps so
                # the in-order PE queue never stalls on GpSimd/DVE latency.
                if u + 2 < NUNIT:
                    bcast(u + 2)

                o_t = o_tiles[u % 3]
                o_s = s_tiles[u % 3]
                # 5 taps x 2 precision splits, PSUM-accumulated (N=512 each).
                # Odd taps read the shifted tile at even (4B-aligned) offsets.
                ys = [psB.tile([FP, 32, W], f32, tag=f"y{h}", name=f"y{h}") for h in range(2)]
                first = True
                for s in range(2):
                    for k in range(K):
                        src_t, kk = (o_t, k) if (k % 2 == 0 or u == 0) else (o_s, k - 1)
                        for h in range(2):
                            nc.tensor.matmul(
                                ys[h][:, :, :],
                                gtab[:, s * K + k, :],
                                src_t[:, h * 32 : (h + 1) * 32, kk : kk + W],
                                start=first,
                                stop=(s == 1 and k == K - 1),
                                skip_group_check=True,
                            )
                        first = False

                # max over the 16 char positions (dense innermost reduce)
                for h in range(2):
                    nc.vector.reduce_max(
                        out=out_sb[:, uu * UNIT + h * 32 : uu * UNIT + (h + 1) * 32],
                        in_=ys[h][0:F, :, :],
                        axis=mybir.AxisListType.X,
                    )

                # stream this unit's result out immediately so the kernel
                # tail only waits on the final 64 tokens
                nc.sync.dma_start(
                    out=out_d[g, :, uu * UNIT : (uu + 1) * UNIT],
                    in_=out_sb[:, uu * UNIT : (uu + 1) * UNIT],
                )

    nc.compile()
    return nc


def make_consts(embed_table, conv_w, conv_b):
    # G[k][v, f] = sum_d E[v, d] * Wc[f, d, k] in float64, split hi/lo fp16
    G = np.einsum(
        "vd,fdk->kvf", embed_table.astype(np.float64), conv_w.astype(np.float64)
    )
    Gf = np.zeros((K, VP, F), np.float64)
    Gf[:, 0:VOCAB, :] = G
    Gf[2, VOCAB, :] = conv_b.astype(np.float64)  # bias rides center tap
    hi = Gf.astype(np.float32).astype(np.float16)
    lo = (Gf - hi.astype(np.float64)).astype(np.float32).astype(np.float16)
    gtab = np.zeros((VP, 2 * K, FP), np.float16)
    gtab[:, 0:K, 0:F] = np.transpose(hi, (1, 0, 2))
    gtab[:, K : 2 * K, 0:F] = np.transpose(lo, (1, 0, 2))
    iota = np.arange(VOCAB, dtype=np.float32).reshape(VOCAB, 1)
    oones = np.ones((1, UNIT * WP), np.float16)
    return gtab, iota, oones


_NC_CACHE = {}

# Test-harness knobs (ignored by normal kernel() use)
TRACE = False
LAST_RESULT = None


def kernel(char_ids, embed_table, conv_w, conv_b):
    global LAST_RESULT
    char_ids = np.asarray(char_ids)
    gtab, iota, oones = make_consts(
        np.asarray(embed_table), np.asarray(conv_w), np.asarray(conv_b)
    )

    if "nc" not in _NC_CACHE:
        _NC_CACHE["nc"] = build_nc()
    nc = _NC_CACHE["nc"]

    in_maps = []
    for c in range(N_CORES):
        shard = np.ascontiguousarray(char_ids[c * NSH : (c + 1) * NSH])
        in_maps.append({"ids": shard, "gtab": gtab, "iota": iota, "oones": oones})

    kwargs = {}
    if TRACE:
        kwargs = dict(trace=True, trace_cores=list(range(N_CORES)))
    res = run_bass_kernel_spmd(nc, in_maps, core_ids=list(range(N_CORES)), **kwargs)
    LAST_RESULT = res

    out = np.empty((N, F), np.float32)
    for c in range(N_CORES):
        o = res.results[c]["out"]  # [NGROUP, F, GROUP]
        out[c * NSH : (c + 1) * NSH] = o.transpose(0, 2, 1).reshape(NSH, F)
    return out
